# revision 3
# baseline (speedup 1.0000x reference)
"""Trainium2 Bass kernel for DigitConvolutionalModel forward pass.

Model: x[B,784] -> 3x3 valid conv (28x28 -> 26x26) -> flatten[676]
       -> Linear(676->200) + ReLU -> Linear(200->10).

Key algebraic optimization: the conv is linear and feeds straight into the
first Linear, so both fold into a single effective weight
W_eff[200,784] = w0 compose conv  (computed once on host, ~1.2 MFLOP).
The device then runs two dense GEMMs per batch shard:
    h = relu(x @ W_eff.T + b0);  out = h @ w1.T + b1

Sharding: pure data parallel over the batch dim across 8 NeuronCores
(4096 rows each). Weights are replicated. No collectives needed (forward
only). Everything on-device runs in a feature-major ("transposed") layout
so the contraction dim always lives on SBUF partitions:
    xT[784,4096] -> hT[200,4096] -> outT[10,4096]
The host pre-transposes x shards and post-transposes the tiny output.
"""

import os
import sys
import types
import contextlib
import numpy as np

for _p in ("/opt/trn_rl_repo", "/root/.axon_site"):
    if os.path.isdir(_p) and _p not in sys.path:
        sys.path.insert(0, _p)

import concourse.bass as bass  # noqa: E402
import concourse.tile as tile  # noqa: E402
import concourse.mybir as mybir  # noqa: E402
from concourse import bacc  # noqa: E402
from concourse.bass_utils import run_bass_kernel_spmd  # noqa: E402

B = 32768
N_CORES = 8
SHARD = B // N_CORES          # 4096
KDIM = 784                    # 28*28 input features (conv folded in)
HID = 200
OUT = 10
CHUNK = 512                   # batch columns processed per matmul group
K_FULL = KDIM // 128          # 6 full 128-partition k-tiles
K_REM = KDIM - K_FULL * 128   # 16 remainder rows
# hidden dim split across PSUM partition tiles: 200 = 128 + 72
M_TILES = [(0, 128), (128, 72)]

# matmul operand dtype: float32 (exact, 4 cyc/row) or float32r
# (1 cyc/row at N>=256, reduced-precision multiplies)
MM_DT = mybir.dt.float32

last_exec_time_ns = None      # set when BASS_KERNEL_PROFILE=1


def _install_ntff_hook():
    """Register the axon NTFF profile hook if the image's antenv lacks it."""
    try:
        from antenv.axon_hooks import get_axon_ntff_profile_hook  # noqa: F401
        return
    except ImportError:
        pass
    try:
        from trn_agent_boot.trn_boot import _ntff_profile_via_ctypes
        hook = _ntff_profile_via_ctypes("/opt/axon/libaxon_pjrt.so")
    except Exception:
        hook = None
    mod = types.ModuleType("antenv.axon_hooks")
    mod.get_axon_ntff_profile_hook = lambda: hook
    mod.set_axon_ntff_profile_hook = lambda h: None
    sys.modules["antenv.axon_hooks"] = mod


def fold_conv_into_fc(conv_w: np.ndarray, w0: np.ndarray) -> np.ndarray:
    """W_eff[200,784] such that x @ W_eff.T == fc1(flatten(conv(x)))."""
    w0v = w0.reshape(HID, 26, 26).astype(np.float64)
    w_img = np.zeros((HID, 28, 28), dtype=np.float64)
    for ki in range(3):
        for kj in range(3):
            w_img[:, ki:ki + 26, kj:kj + 26] += w0v * np.float64(conv_w[ki, kj])
    return w_img.reshape(HID, KDIM).astype(np.float32)


def build_program():
    nc = bacc.Bacc("TRN2", target_bir_lowering=False, debug=False)
    f32 = mybir.dt.float32

    xT = nc.declare_dram_parameter("xT", [KDIM, SHARD], MM_DT, isOutput=False)
    w0t = nc.declare_dram_parameter("w0t", [KDIM, HID], MM_DT, isOutput=False)
    b0 = nc.declare_dram_parameter("b0", [HID, 1], f32, isOutput=False)
    w1t = nc.declare_dram_parameter("w1t", [HID, OUT], MM_DT, isOutput=False)
    b1 = nc.declare_dram_parameter("b1", [OUT, 1], f32, isOutput=False)
    out = nc.declare_dram_parameter("out", [OUT, SHARD], f32, isOutput=True)

    n_chunks = SHARD // CHUNK
    relu = mybir.ActivationFunctionType.Relu
    ident = mybir.ActivationFunctionType.Identity

    with tile.TileContext(nc) as tc:
        with (
            tc.tile_pool(name="weights", bufs=1) as wpool,
            tc.tile_pool(name="xin", bufs=3) as xpool,
            tc.tile_pool(name="hbuf", bufs=3) as hpool,
            tc.tile_pool(name="obuf", bufs=3) as opool,
            tc.tile_pool(name="psum", bufs=2, space=bass.MemorySpace.PSUM) as pp,
        ):
            # ---- replicated weights, loaded once ----
            w0_main = wpool.tile([128, K_FULL, HID], MM_DT)   # k rows 0..767
            nc.sync.dma_start(
                w0_main[:],
                w0t[0:K_FULL * 128, :].rearrange("(a p) m -> p a m", p=128),
            )
            w0_rem = wpool.tile([K_REM, HID], MM_DT)          # k rows 768..783
            nc.sync.dma_start(w0_rem[:], w0t[K_FULL * 128:KDIM, :])
            w1_a = wpool.tile([128, OUT], MM_DT)
            nc.sync.dma_start(w1_a[:], w1t[0:128, :])
            w1_b = wpool.tile([72, OUT], MM_DT)
            nc.sync.dma_start(w1_b[:], w1t[128:HID, :])
            b0_a = wpool.tile([128, 1], f32)
            nc.sync.dma_start(b0_a[:], b0[0:128, :])
            b0_b = wpool.tile([72, 1], f32)
            nc.sync.dma_start(b0_b[:], b0[128:HID, :])
            b1_t = wpool.tile([OUT, 1], f32)
            nc.sync.dma_start(b1_t[:], b1[:])
            b0_tiles = [b0_a, b0_b]
            w1_tiles = [w1_a, w1_b]

            for c in range(n_chunks):
                n0 = c * CHUNK
                # ---- load xT chunk: [784, CHUNK] as one big + one small tile
                xk = xpool.tile([128, K_FULL, CHUNK], MM_DT, tag="xk")
                nc.sync.dma_start(
                    xk[:],
                    xT[0:K_FULL * 128, n0:n0 + CHUNK].rearrange(
                        "(a p) n -> p a n", p=128
                    ),
                )
                xr = xpool.tile([K_REM, CHUNK], MM_DT, tag="xr")
                nc.sync.dma_start(xr[:], xT[K_FULL * 128:KDIM, n0:n0 + CHUNK])

                # ---- layer 1: hT[m0:m0+dm, chunk] accumulated over 7 k-tiles
                h_tiles = []
                for mi, (m0, dm) in enumerate(M_TILES):
                    h_ps = pp.tile([dm, CHUNK], f32, tag=f"hps{mi}")
                    for a in range(K_FULL):
                        nc.tensor.matmul(
                            h_ps[:],
                            w0_main[:, a, m0:m0 + dm],
                            xk[:, a, :],
                            start=(a == 0),
                            stop=False,
                        )
                    nc.tensor.matmul(
                        h_ps[:],
                        w0_rem[:, m0:m0 + dm],
                        xr[:],
                        start=False,
                        stop=True,
                    )
                    h_sb = hpool.tile([dm, CHUNK], MM_DT, tag=f"h{mi}")
                    nc.scalar.activation(h_sb[:], h_ps[:], relu, bias=b0_tiles[mi][:])
                    h_tiles.append(h_sb)

                # ---- layer 2: outT[10, chunk] accumulated over 2 k-tiles
                o_ps = pp.tile([OUT, CHUNK], f32, tag="ops")
                for mi in range(2):
                    nc.tensor.matmul(
                        o_ps[:],
                        w1_tiles[mi][:],
                        h_tiles[mi][:],
                        start=(mi == 0),
                        stop=(mi == 1),
                    )
                o_sb = opool.tile([OUT, CHUNK], f32, tag="osb")
                nc.scalar.activation(o_sb[:], o_ps[:], ident, bias=b1_t[:])
                nc.sync.dma_start(out[:, n0:n0 + CHUNK], o_sb[:])

    nc.compile()
    return nc


_program_cache = {}


def _get_program():
    key = (MM_DT, CHUNK)
    if key not in _program_cache:
        _program_cache[key] = build_program()
    return _program_cache[key]


def kernel(**inputs: np.ndarray) -> np.ndarray:
    x = np.asarray(inputs["x"], dtype=np.float32)
    conv_w = np.asarray(inputs["conv_w"], dtype=np.float32)
    w0 = np.asarray(inputs["w0"], dtype=np.float32)
    b0 = np.asarray(inputs["b0"], dtype=np.float32)
    w1 = np.asarray(inputs["w1"], dtype=np.float32)
    b1 = np.asarray(inputs["b1"], dtype=np.float32)

    w_eff = fold_conv_into_fc(conv_w, w0)
    w0t_np = np.ascontiguousarray(w_eff.T)           # [784, 200]
    w1t_np = np.ascontiguousarray(w1.T)              # [200, 10]
    b0_np = np.ascontiguousarray(b0.reshape(HID, 1))
    b1_np = np.ascontiguousarray(b1.reshape(OUT, 1))

    in_maps = []
    for i in range(N_CORES):
        xs = x[i * SHARD:(i + 1) * SHARD]            # [4096, 784]
        in_maps.append({
            "xT": np.ascontiguousarray(xs.T),        # [784, 4096]
            "w0t": w0t_np,
            "b0": b0_np,
            "w1t": w1t_np,
            "b1": b1_np,
        })

    nc = _get_program()

    profile = os.environ.get("BASS_KERNEL_PROFILE", "0") == "1"
    kwargs = {}
    if profile:
        _install_ntff_hook()
        kwargs = dict(trace=True, tmpdir=os.environ.get("BASS_KERNEL_TRACE_DIR"))
    res = run_bass_kernel_spmd(nc, in_maps, core_ids=list(range(N_CORES)), **kwargs)

    global last_exec_time_ns
    last_exec_time_ns = res.exec_time_ns

    out = np.empty((B, OUT), dtype=np.float32)
    for i in range(N_CORES):
        out[i * SHARD:(i + 1) * SHARD] = res.results[i]["out"].T
    return out


# revision 4
# speedup vs baseline: 1.4653x; 1.4653x over previous
"""Trainium2 Bass kernel for DigitConvolutionalModel forward pass.

Model: x[B,784] -> 3x3 valid conv (28x28 -> 26x26) -> flatten[676]
       -> Linear(676->200) + ReLU -> Linear(200->10).

Key algebraic optimization: the conv is linear and feeds straight into the
first Linear, so both fold into a single effective weight
W_eff[200,784] = w0 compose conv  (computed once on host, ~1.2 MFLOP).
The device then runs two dense GEMMs per batch shard:
    h = relu(x @ W_eff.T + b0);  out = h @ w1.T + b1

Sharding: pure data parallel over the batch dim across 8 NeuronCores
(4096 rows each). Weights are replicated. No collectives needed (forward
only). Everything on-device runs in a feature-major ("transposed") layout
so the contraction dim always lives on SBUF partitions:
    xT[784,4096] -> hT[200,4096] -> outT[10,4096]
The host pre-transposes x shards and post-transposes the tiny output.
"""

import os
import sys
import types
import contextlib
import numpy as np

for _p in ("/opt/trn_rl_repo", "/root/.axon_site"):
    if os.path.isdir(_p) and _p not in sys.path:
        sys.path.insert(0, _p)

import concourse.bass as bass  # noqa: E402
import concourse.tile as tile  # noqa: E402
import concourse.mybir as mybir  # noqa: E402
from concourse import bacc  # noqa: E402
from concourse.bass_utils import run_bass_kernel_spmd  # noqa: E402

B = 32768
N_CORES = 8
SHARD = B // N_CORES          # 4096
KDIM = 784                    # 28*28 input features (conv folded in)
HID = 200
OUT = 10
CHUNK = 512                   # batch columns processed per matmul group
K_FULL = KDIM // 128          # 6 full 128-partition k-tiles
K_REM = KDIM - K_FULL * 128   # 16 remainder rows
# hidden dim split across PSUM partition tiles: 200 = 128 + 72
M_TILES = [(0, 128), (128, 72)]

# matmul operand dtype: float32 (exact, 4 cyc/row) or float32r
# (1 cyc/row at N>=256, reduced-precision multiplies)
MM_DT = mybir.dt.float32r

last_exec_time_ns = None      # set when BASS_KERNEL_PROFILE=1


def _install_ntff_hook():
    """Register the axon NTFF profile hook if the image's antenv lacks it."""
    try:
        from antenv.axon_hooks import get_axon_ntff_profile_hook  # noqa: F401
        return
    except ImportError:
        pass
    try:
        from trn_agent_boot.trn_boot import _ntff_profile_via_ctypes
        hook = _ntff_profile_via_ctypes("/opt/axon/libaxon_pjrt.so")
    except Exception:
        hook = None
    mod = types.ModuleType("antenv.axon_hooks")
    mod.get_axon_ntff_profile_hook = lambda: hook
    mod.set_axon_ntff_profile_hook = lambda h: None
    sys.modules["antenv.axon_hooks"] = mod


def fold_conv_into_fc(conv_w: np.ndarray, w0: np.ndarray) -> np.ndarray:
    """W_eff[200,784] such that x @ W_eff.T == fc1(flatten(conv(x)))."""
    w0v = w0.reshape(HID, 26, 26).astype(np.float64)
    w_img = np.zeros((HID, 28, 28), dtype=np.float64)
    for ki in range(3):
        for kj in range(3):
            w_img[:, ki:ki + 26, kj:kj + 26] += w0v * np.float64(conv_w[ki, kj])
    return w_img.reshape(HID, KDIM).astype(np.float32)


def build_program():
    nc = bacc.Bacc("TRN2", target_bir_lowering=False, debug=False)
    f32 = mybir.dt.float32

    xT = nc.declare_dram_parameter("xT", [KDIM, SHARD], MM_DT, isOutput=False)
    w0t = nc.declare_dram_parameter("w0t", [KDIM, HID], MM_DT, isOutput=False)
    b0 = nc.declare_dram_parameter("b0", [HID, 1], f32, isOutput=False)
    w1t = nc.declare_dram_parameter("w1t", [HID, OUT], MM_DT, isOutput=False)
    b1 = nc.declare_dram_parameter("b1", [OUT, 1], f32, isOutput=False)
    out = nc.declare_dram_parameter("out", [OUT, SHARD], f32, isOutput=True)

    n_chunks = SHARD // CHUNK
    relu = mybir.ActivationFunctionType.Relu
    ident = mybir.ActivationFunctionType.Identity

    with tile.TileContext(nc) as tc:
        with (
            tc.tile_pool(name="weights", bufs=1) as wpool,
            tc.tile_pool(name="xin", bufs=3) as xpool,
            tc.tile_pool(name="hbuf", bufs=3) as hpool,
            tc.tile_pool(name="obuf", bufs=3) as opool,
            tc.tile_pool(name="psum", bufs=2, space=bass.MemorySpace.PSUM) as pp,
        ):
            # ---- replicated weights, loaded once ----
            w0_main = wpool.tile([128, K_FULL, HID], MM_DT)   # k rows 0..767
            nc.sync.dma_start(
                w0_main[:],
                w0t[0:K_FULL * 128, :].rearrange("(a p) m -> p a m", p=128),
            )
            w0_rem = wpool.tile([K_REM, HID], MM_DT)          # k rows 768..783
            nc.sync.dma_start(w0_rem[:], w0t[K_FULL * 128:KDIM, :])
            w1_a = wpool.tile([128, OUT], MM_DT)
            nc.sync.dma_start(w1_a[:], w1t[0:128, :])
            w1_b = wpool.tile([72, OUT], MM_DT)
            nc.sync.dma_start(w1_b[:], w1t[128:HID, :])
            b0_a = wpool.tile([128, 1], f32)
            nc.sync.dma_start(b0_a[:], b0[0:128, :])
            b0_b = wpool.tile([72, 1], f32)
            nc.sync.dma_start(b0_b[:], b0[128:HID, :])
            b1_t = wpool.tile([OUT, 1], f32)
            nc.sync.dma_start(b1_t[:], b1[:])
            b0_tiles = [b0_a, b0_b]
            w1_tiles = [w1_a, w1_b]

            for c in range(n_chunks):
                n0 = c * CHUNK
                # ---- load xT chunk: [784, CHUNK] as one big + one small tile
                xk = xpool.tile([128, K_FULL, CHUNK], MM_DT, tag="xk")
                nc.sync.dma_start(
                    xk[:],
                    xT[0:K_FULL * 128, n0:n0 + CHUNK].rearrange(
                        "(a p) n -> p a n", p=128
                    ),
                )
                xr = xpool.tile([K_REM, CHUNK], MM_DT, tag="xr")
                nc.sync.dma_start(xr[:], xT[K_FULL * 128:KDIM, n0:n0 + CHUNK])

                # ---- layer 1: hT[m0:m0+dm, chunk] accumulated over 7 k-tiles
                h_tiles = []
                for mi, (m0, dm) in enumerate(M_TILES):
                    h_ps = pp.tile([dm, CHUNK], f32, tag=f"hps{mi}")
                    for a in range(K_FULL):
                        nc.tensor.matmul(
                            h_ps[:],
                            w0_main[:, a, m0:m0 + dm],
                            xk[:, a, :],
                            start=(a == 0),
                            stop=False,
                        )
                    nc.tensor.matmul(
                        h_ps[:],
                        w0_rem[:, m0:m0 + dm],
                        xr[:],
                        start=False,
                        stop=True,
                    )
                    h_sb = hpool.tile([dm, CHUNK], MM_DT, tag=f"h{mi}")
                    nc.scalar.activation(h_sb[:], h_ps[:], relu, bias=b0_tiles[mi][:])
                    h_tiles.append(h_sb)

                # ---- layer 2: outT[10, chunk] accumulated over 2 k-tiles
                o_ps = pp.tile([OUT, CHUNK], f32, tag="ops")
                for mi in range(2):
                    nc.tensor.matmul(
                        o_ps[:],
                        w1_tiles[mi][:],
                        h_tiles[mi][:],
                        start=(mi == 0),
                        stop=(mi == 1),
                    )
                o_sb = opool.tile([OUT, CHUNK], f32, tag="osb")
                nc.scalar.activation(o_sb[:], o_ps[:], ident, bias=b1_t[:])
                nc.sync.dma_start(out[:, n0:n0 + CHUNK], o_sb[:])

    nc.compile()
    return nc


_program_cache = {}


def _get_program():
    key = (MM_DT, CHUNK)
    if key not in _program_cache:
        _program_cache[key] = build_program()
    return _program_cache[key]


def kernel(**inputs: np.ndarray) -> np.ndarray:
    x = np.asarray(inputs["x"], dtype=np.float32)
    conv_w = np.asarray(inputs["conv_w"], dtype=np.float32)
    w0 = np.asarray(inputs["w0"], dtype=np.float32)
    b0 = np.asarray(inputs["b0"], dtype=np.float32)
    w1 = np.asarray(inputs["w1"], dtype=np.float32)
    b1 = np.asarray(inputs["b1"], dtype=np.float32)

    w_eff = fold_conv_into_fc(conv_w, w0)
    w0t_np = np.ascontiguousarray(w_eff.T)           # [784, 200]
    w1t_np = np.ascontiguousarray(w1.T)              # [200, 10]
    b0_np = np.ascontiguousarray(b0.reshape(HID, 1))
    b1_np = np.ascontiguousarray(b1.reshape(OUT, 1))

    in_maps = []
    for i in range(N_CORES):
        xs = x[i * SHARD:(i + 1) * SHARD]            # [4096, 784]
        in_maps.append({
            "xT": np.ascontiguousarray(xs.T),        # [784, 4096]
            "w0t": w0t_np,
            "b0": b0_np,
            "w1t": w1t_np,
            "b1": b1_np,
        })

    nc = _get_program()

    profile = os.environ.get("BASS_KERNEL_PROFILE", "0") == "1"
    kwargs = {}
    if profile:
        _install_ntff_hook()
        kwargs = dict(trace=True, tmpdir=os.environ.get("BASS_KERNEL_TRACE_DIR"))
    res = run_bass_kernel_spmd(nc, in_maps, core_ids=list(range(N_CORES)), **kwargs)

    global last_exec_time_ns
    last_exec_time_ns = res.exec_time_ns

    out = np.empty((B, OUT), dtype=np.float32)
    for i in range(N_CORES):
        out[i * SHARD:(i + 1) * SHARD] = res.results[i]["out"].T
    return out


# revision 6
# speedup vs baseline: 1.5938x; 1.0877x over previous
"""Trainium2 Bass kernel for DigitConvolutionalModel forward pass.

Model: x[B,784] -> 3x3 valid conv (28x28 -> 26x26) -> flatten[676]
       -> Linear(676->200) + ReLU -> Linear(200->10).

Key algebraic optimization: the conv is linear and feeds straight into the
first Linear, so both fold into a single effective weight
W_eff[200,784] = w0 compose conv  (computed once on host, ~1.2 MFLOP).
The device then runs two dense GEMMs per batch shard:
    h = relu(x @ W_eff.T + b0);  out = h @ w1.T + b1

Sharding: pure data parallel over the batch dim across 8 NeuronCores
(4096 rows each). Weights are replicated. No collectives needed (forward
only). Everything on-device runs in a feature-major ("transposed") layout
so the contraction dim always lives on SBUF partitions:
    xT[784,4096] -> hT[200,4096] -> outT[10,4096]
The host pre-transposes x shards (and pre-tiles them so every DMA reads
long contiguous runs per SBUF partition) and post-transposes the tiny
output.
"""

import os
import sys
import types
import numpy as np

for _p in ("/opt/trn_rl_repo", "/root/.axon_site"):
    if os.path.isdir(_p) and _p not in sys.path:
        sys.path.insert(0, _p)

import concourse.bass as bass  # noqa: E402
import concourse.tile as tile  # noqa: E402
import concourse.mybir as mybir  # noqa: E402
from concourse import bacc  # noqa: E402
from concourse.bass_utils import run_bass_kernel_spmd  # noqa: E402

B = 32768
N_CORES = 8
SHARD = B // N_CORES          # 4096
KDIM = 784                    # 28*28 input features (conv folded in)
HID = 200
OUT = 10
CHUNK = 512                   # batch columns per matmul (moving free dim)
PAIR = 2                      # chunks processed per weight pass (LDW reuse)
K_FULL = KDIM // 128          # 6 full 128-partition k-tiles
K_REM = KDIM - K_FULL * 128   # 16 remainder rows
M_TILES = [(0, 128), (128, 72)]  # hidden 200 = 128 + 72 PSUM partition tiles

# matmul operand dtype: float32 (exact, 4 cyc/row) or float32r
# (1 cyc/row at N>=256, reduced-precision multiplies, rel err ~2e-4)
MM_DT = mybir.dt.float32r

last_exec_time_ns = None      # set when BASS_KERNEL_PROFILE=1


def _install_ntff_hook():
    """Register the axon NTFF profile hook if the image's antenv lacks it."""
    try:
        from antenv.axon_hooks import get_axon_ntff_profile_hook  # noqa: F401
        return
    except ImportError:
        pass
    try:
        from trn_agent_boot.trn_boot import _ntff_profile_via_ctypes
        hook = _ntff_profile_via_ctypes("/opt/axon/libaxon_pjrt.so")
    except Exception:
        hook = None
    mod = types.ModuleType("antenv.axon_hooks")
    mod.get_axon_ntff_profile_hook = lambda: hook
    mod.set_axon_ntff_profile_hook = lambda h: None
    sys.modules["antenv.axon_hooks"] = mod


def fold_conv_into_fc(conv_w: np.ndarray, w0: np.ndarray) -> np.ndarray:
    """W_eff[200,784] such that x @ W_eff.T == fc1(flatten(conv(x)))."""
    w0v = w0.reshape(HID, 26, 26).astype(np.float64)
    w_img = np.zeros((HID, 28, 28), dtype=np.float64)
    for ki in range(3):
        for kj in range(3):
            w_img[:, ki:ki + 26, kj:kj + 26] += w0v * np.float64(conv_w[ki, kj])
    return w_img.reshape(HID, KDIM).astype(np.float32)


def shard_layout(xs: np.ndarray):
    """Pre-tile one x shard [4096, 784] for DMA-friendly loads.

    Returns (xm, xr):
      xm[c, p, a, n] = x[c*CHUNK+n, a*128+p]   shape [8, 128, 6, 512]
      xr[c, p, n]    = x[c*CHUNK+n, 768+p]     shape [8, 16, 512]
    so each SBUF partition line is one contiguous (a, n) run of 12 KB.
    """
    n_chunks = SHARD // CHUNK
    xsv = xs.reshape(n_chunks, CHUNK, KDIM)
    xm = np.ascontiguousarray(
        xsv[:, :, :K_FULL * 128]
        .reshape(n_chunks, CHUNK, K_FULL, 128)
        .transpose(0, 3, 2, 1)
    )
    xr = np.ascontiguousarray(
        xsv[:, :, K_FULL * 128:].transpose(0, 2, 1)
    )
    return xm, xr


def build_program():
    nc = bacc.Bacc("TRN2", target_bir_lowering=False, debug=False)
    f32 = mybir.dt.float32
    n_chunks = SHARD // CHUNK

    xm_d = nc.declare_dram_parameter(
        "xm", [n_chunks, 128, K_FULL, CHUNK], MM_DT, isOutput=False)
    xr_d = nc.declare_dram_parameter(
        "xr", [n_chunks, K_REM, CHUNK], MM_DT, isOutput=False)
    w0t = nc.declare_dram_parameter("w0t", [KDIM, HID], MM_DT, isOutput=False)
    b0 = nc.declare_dram_parameter("b0", [HID, 1], f32, isOutput=False)
    w1t = nc.declare_dram_parameter("w1t", [HID, OUT], MM_DT, isOutput=False)
    b1 = nc.declare_dram_parameter("b1", [OUT, 1], f32, isOutput=False)
    out = nc.declare_dram_parameter("out", [OUT, SHARD], f32, isOutput=True)

    relu = mybir.ActivationFunctionType.Relu
    ident = mybir.ActivationFunctionType.Identity

    with tile.TileContext(nc) as tc:
        with (
            tc.tile_pool(name="weights", bufs=1) as wpool,
            tc.tile_pool(name="xin", bufs=3) as xpool,
            tc.tile_pool(name="hbuf", bufs=2) as hpool,
            tc.tile_pool(name="obuf", bufs=4) as opool,
            tc.tile_pool(name="psum", bufs=1, space=bass.MemorySpace.PSUM) as pp,
            tc.tile_pool(name="opsum", bufs=4, space=bass.MemorySpace.PSUM) as op,
        ):
            # ---- replicated weights, loaded once ----
            w0_main = wpool.tile([128, K_FULL, HID], MM_DT)   # k rows 0..767
            nc.sync.dma_start(
                w0_main[:],
                w0t[0:K_FULL * 128, :].rearrange("(a p) m -> p a m", p=128),
            )
            w0_rem = wpool.tile([K_REM, HID], MM_DT)          # k rows 768..783
            nc.sync.dma_start(w0_rem[:], w0t[K_FULL * 128:KDIM, :])
            w1_a = wpool.tile([128, OUT], MM_DT)
            nc.sync.dma_start(w1_a[:], w1t[0:128, :])
            w1_b = wpool.tile([72, OUT], MM_DT)
            nc.sync.dma_start(w1_b[:], w1t[128:HID, :])
            b0_a = wpool.tile([128, 1], f32)
            nc.sync.dma_start(b0_a[:], b0[0:128, :])
            b0_b = wpool.tile([72, 1], f32)
            nc.sync.dma_start(b0_b[:], b0[128:HID, :])
            b1_t = wpool.tile([OUT, 1], f32)
            nc.sync.dma_start(b1_t[:], b1[:])
            b0_tiles = [b0_a, b0_b]
            w1_tiles = [w1_a, w1_b]

            for g in range(n_chunks // PAIR):
                chunks = [g * PAIR + j for j in range(PAIR)]
                xks, xrs = [], []
                for c in chunks:
                    xk = xpool.tile([128, K_FULL, CHUNK], MM_DT, tag="xk")
                    nc.sync.dma_start(xk[:], xm_d[c])
                    xr = xpool.tile([K_REM, CHUNK], MM_DT, tag="xr")
                    nc.sync.dma_start(xr[:], xr_d[c])
                    xks.append(xk)
                    xrs.append(xr)

                # layer 1: same stationary weight feeds PAIR moving chunks
                h_ps = [[pp.tile([dm, CHUNK], f32, tag=f"hps{j}{mi}",
                                 name=f"hps_{g}_{j}_{mi}")
                         for mi, (m0, dm) in enumerate(M_TILES)]
                        for j in range(PAIR)]
                for mi, (m0, dm) in enumerate(M_TILES):
                    for a in range(K_FULL):
                        for j in range(PAIR):
                            nc.tensor.matmul(
                                h_ps[j][mi][:],
                                w0_main[:, a, m0:m0 + dm],
                                xks[j][:, a, :],
                                start=(a == 0),
                                stop=False,
                            )
                    for j in range(PAIR):
                        nc.tensor.matmul(
                            h_ps[j][mi][:],
                            w0_rem[:, m0:m0 + dm],
                            xrs[j][:],
                            start=False,
                            stop=True,
                        )

                # relu + bias (scalar engine), then layer 2
                for j in range(PAIR):
                    h_tiles = []
                    for mi, (m0, dm) in enumerate(M_TILES):
                        h_sb = hpool.tile([dm, CHUNK], MM_DT, tag=f"h{mi}")
                        nc.scalar.activation(
                            h_sb[:], h_ps[j][mi][:], relu, bias=b0_tiles[mi][:])
                        h_tiles.append(h_sb)
                    o_ps = op.tile([OUT, CHUNK], f32, tag="ops")
                    for mi in range(2):
                        nc.tensor.matmul(
                            o_ps[:],
                            w1_tiles[mi][:],
                            h_tiles[mi][:],
                            start=(mi == 0),
                            stop=(mi == 1),
                        )
                    o_sb = opool.tile([OUT, CHUNK], f32, tag="osb")
                    nc.scalar.activation(o_sb[:], o_ps[:], ident, bias=b1_t[:])
                    n0 = chunks[j] * CHUNK
                    nc.sync.dma_start(out[:, n0:n0 + CHUNK], o_sb[:])

    nc.compile()
    return nc


_program_cache = {}


def _get_program():
    key = (MM_DT, CHUNK, PAIR)
    if key not in _program_cache:
        _program_cache[key] = build_program()
    return _program_cache[key]


def kernel(**inputs: np.ndarray) -> np.ndarray:
    x = np.asarray(inputs["x"], dtype=np.float32)
    conv_w = np.asarray(inputs["conv_w"], dtype=np.float32)
    w0 = np.asarray(inputs["w0"], dtype=np.float32)
    b0 = np.asarray(inputs["b0"], dtype=np.float32)
    w1 = np.asarray(inputs["w1"], dtype=np.float32)
    b1 = np.asarray(inputs["b1"], dtype=np.float32)

    w_eff = fold_conv_into_fc(conv_w, w0)
    w0t_np = np.ascontiguousarray(w_eff.T)           # [784, 200]
    w1t_np = np.ascontiguousarray(w1.T)              # [200, 10]
    b0_np = np.ascontiguousarray(b0.reshape(HID, 1))
    b1_np = np.ascontiguousarray(b1.reshape(OUT, 1))

    in_maps = []
    for i in range(N_CORES):
        xm, xr = shard_layout(x[i * SHARD:(i + 1) * SHARD])
        in_maps.append({
            "xm": xm,
            "xr": xr,
            "w0t": w0t_np,
            "b0": b0_np,
            "w1t": w1t_np,
            "b1": b1_np,
        })

    nc = _get_program()

    profile = os.environ.get("BASS_KERNEL_PROFILE", "0") == "1"
    kwargs = {}
    if profile:
        _install_ntff_hook()
        kwargs = dict(trace=True, tmpdir=os.environ.get("BASS_KERNEL_TRACE_DIR"))
    res = run_bass_kernel_spmd(nc, in_maps, core_ids=list(range(N_CORES)), **kwargs)

    global last_exec_time_ns
    last_exec_time_ns = res.exec_time_ns

    out = np.empty((B, OUT), dtype=np.float32)
    for i in range(N_CORES):
        out[i * SHARD:(i + 1) * SHARD] = res.results[i]["out"].T
    return out


# revision 9
# speedup vs baseline: 1.8155x; 1.1391x over previous
"""Trainium2 Bass kernel for DigitConvolutionalModel forward pass.

Model: x[B,784] -> 3x3 valid conv (28x28 -> 26x26) -> flatten[676]
       -> Linear(676->200) + ReLU -> Linear(200->10).

Key algebraic optimization: the conv is linear and feeds straight into the
first Linear, so both fold into a single effective weight
W_eff[200,784] = w0 compose conv  (computed once on host, ~1.2 MFLOP).
The device then runs two dense GEMMs per batch shard:
    h = relu(x @ W_eff.T + b0);  out = h @ w1.T + b1

Sharding: pure data parallel over the batch dim across 8 NeuronCores
(4096 rows each). Weights are replicated. No collectives needed (forward
only). Everything on-device runs in a feature-major ("transposed") layout
so the contraction dim always lives on SBUF partitions:
    xT[784,4096] -> hT[200,4096] -> outT[10,4096]
The host pre-transposes x shards (and pre-tiles them so every DMA reads
long contiguous runs per SBUF partition) and post-transposes the tiny
output.
"""

import os
import sys
import types
import numpy as np

for _p in ("/opt/trn_rl_repo", "/root/.axon_site"):
    if os.path.isdir(_p) and _p not in sys.path:
        sys.path.insert(0, _p)

import concourse.bass as bass  # noqa: E402
import concourse.tile as tile  # noqa: E402
import concourse.mybir as mybir  # noqa: E402
from concourse import bacc  # noqa: E402
from concourse.bass_utils import run_bass_kernel_spmd  # noqa: E402

B = 32768
N_CORES = 8
SHARD = B // N_CORES          # 4096
KDIM = 784                    # 28*28 input features (conv folded in)
HID = 200
OUT = 10
CHUNK = 512                   # batch columns per matmul (moving free dim)
K_FULL = KDIM // 128          # 6 full 128-partition k-tiles
K_REM = KDIM - K_FULL * 128   # 16 remainder rows
M_TILES = [(0, 128), (128, 72)]  # hidden 200 = 128 + 72 PSUM partition tiles

# matmul operand dtype: float32 (exact, 4 cyc/row) or float32r
# (1 cyc/row at N>=256, reduced-precision multiplies, rel err ~2e-4)
MM_DT = mybir.dt.float32r

last_exec_time_ns = None      # set when BASS_KERNEL_PROFILE=1


def _install_ntff_hook():
    """Register the axon NTFF profile hook if the image's antenv lacks it."""
    try:
        from antenv.axon_hooks import get_axon_ntff_profile_hook  # noqa: F401
        return
    except ImportError:
        pass
    try:
        from trn_agent_boot.trn_boot import _ntff_profile_via_ctypes
        hook = _ntff_profile_via_ctypes("/opt/axon/libaxon_pjrt.so")
    except Exception:
        hook = None
    mod = types.ModuleType("antenv.axon_hooks")
    mod.get_axon_ntff_profile_hook = lambda: hook
    mod.set_axon_ntff_profile_hook = lambda h: None
    sys.modules["antenv.axon_hooks"] = mod


def fold_conv_into_fc(conv_w: np.ndarray, w0: np.ndarray) -> np.ndarray:
    """W_eff[200,784] such that x @ W_eff.T == fc1(flatten(conv(x)))."""
    w0v = w0.reshape(HID, 26, 26).astype(np.float64)
    w_img = np.zeros((HID, 28, 28), dtype=np.float64)
    for ki in range(3):
        for kj in range(3):
            w_img[:, ki:ki + 26, kj:kj + 26] += w0v * np.float64(conv_w[ki, kj])
    return w_img.reshape(HID, KDIM).astype(np.float32)


def shard_layout(xs: np.ndarray):
    """Pre-tile one x shard [4096, 784] for DMA-friendly loads.

    Returns (xm, xr):
      xm[c, p, a, n] = x[c*CHUNK+n, a*128+p]   shape [8, 128, 6, 512]
      xr[c, p, n]    = x[c*CHUNK+n, 768+p]     shape [8, 16, 512]
    so each SBUF partition line is one contiguous (a, n) run of 12 KB.
    """
    n_chunks = SHARD // CHUNK
    xsv = xs.reshape(n_chunks, CHUNK, KDIM)
    xm = np.ascontiguousarray(
        xsv[:, :, :K_FULL * 128]
        .reshape(n_chunks, CHUNK, K_FULL, 128)
        .transpose(0, 3, 2, 1)
    )
    xr = np.ascontiguousarray(
        xsv[:, :, K_FULL * 128:].transpose(0, 2, 1)
    )
    return xm, xr


def build_program():
    nc = bacc.Bacc("TRN2", target_bir_lowering=False, debug=False)
    f32 = mybir.dt.float32
    n_chunks = SHARD // CHUNK

    xm_d = nc.declare_dram_parameter(
        "xm", [n_chunks, 128, K_FULL, CHUNK], MM_DT, isOutput=False)
    xr_d = nc.declare_dram_parameter(
        "xr", [n_chunks, K_REM, CHUNK], MM_DT, isOutput=False)
    w0t = nc.declare_dram_parameter("w0t", [KDIM, HID], MM_DT, isOutput=False)
    b0 = nc.declare_dram_parameter("b0", [HID, 1], f32, isOutput=False)
    w1t = nc.declare_dram_parameter("w1t", [HID, OUT], MM_DT, isOutput=False)
    b1 = nc.declare_dram_parameter("b1", [OUT, 1], f32, isOutput=False)
    out = nc.declare_dram_parameter("out", [OUT, SHARD], f32, isOutput=True)

    relu = mybir.ActivationFunctionType.Relu
    ident = mybir.ActivationFunctionType.Identity

    with tile.TileContext(nc) as tc:
        with (
            tc.tile_pool(name="weights", bufs=1) as wpool,
            tc.tile_pool(name="xin", bufs=5) as xpool,
            tc.tile_pool(name="hbuf", bufs=2) as hpool,
            tc.tile_pool(name="obuf", bufs=4) as opool,
            tc.tile_pool(name="psum", bufs=2, space=bass.MemorySpace.PSUM) as pp,
            tc.tile_pool(name="opsum", bufs=2, space=bass.MemorySpace.PSUM) as op,
        ):
            # ---- replicated weights, loaded once (SWDGE; keeps the two
            # HWDGE rings free for the first x-chunk loads) ----
            w0_main = wpool.tile([128, K_FULL, HID], MM_DT)   # k rows 0..767
            nc.gpsimd.dma_start(
                w0_main[:],
                w0t[0:K_FULL * 128, :].rearrange("(a p) m -> p a m", p=128),
            )
            w0_rem = wpool.tile([K_REM, HID], MM_DT)          # k rows 768..783
            nc.gpsimd.dma_start(w0_rem[:], w0t[K_FULL * 128:KDIM, :])
            w1_a = wpool.tile([128, OUT], MM_DT)
            nc.gpsimd.dma_start(w1_a[:], w1t[0:128, :])
            w1_b = wpool.tile([72, OUT], MM_DT)
            nc.gpsimd.dma_start(w1_b[:], w1t[128:HID, :])
            b0_a = wpool.tile([128, 1], f32)
            nc.gpsimd.dma_start(b0_a[:], b0[0:128, :])
            b0_b = wpool.tile([72, 1], f32)
            nc.gpsimd.dma_start(b0_b[:], b0[128:HID, :])
            b1_t = wpool.tile([OUT, 1], f32)
            nc.gpsimd.dma_start(b1_t[:], b1[:])
            b0_tiles = [b0_a, b0_b]
            w1_tiles = [w1_a, w1_b]

            for c in range(n_chunks):
                # alternate x loads across the two HWDGE rings (SP / ACT)
                dma_eng = nc.sync if c % 2 == 0 else nc.scalar
                xk = xpool.tile([128, K_FULL, CHUNK], MM_DT, tag="xk")
                dma_eng.dma_start(xk[:], xm_d[c])
                xr = xpool.tile([K_REM, CHUNK], MM_DT, tag="xr")
                dma_eng.dma_start(xr[:], xr_d[c])

                # layer 1: hT[m0:m0+dm, chunk] accumulated over 7 k-tiles
                h_tiles = []
                for mi, (m0, dm) in enumerate(M_TILES):
                    h_ps = pp.tile([dm, CHUNK], f32, tag=f"hps{mi}",
                                   name=f"hps_{c}_{mi}")
                    for a in range(K_FULL):
                        nc.tensor.matmul(
                            h_ps[:],
                            w0_main[:, a, m0:m0 + dm],
                            xk[:, a, :],
                            start=(a == 0),
                            stop=False,
                        )
                    nc.tensor.matmul(
                        h_ps[:],
                        w0_rem[:, m0:m0 + dm],
                        xr[:],
                        start=False,
                        stop=True,
                    )
                    h_sb = hpool.tile([dm, CHUNK], MM_DT, tag=f"h{mi}")
                    nc.scalar.activation(
                        h_sb[:], h_ps[:], relu, bias=b0_tiles[mi][:])
                    h_tiles.append(h_sb)

                # layer 2: outT[10, chunk] accumulated over 2 k-tiles
                o_ps = op.tile([OUT, CHUNK], f32, tag="ops")
                for mi in range(2):
                    nc.tensor.matmul(
                        o_ps[:],
                        w1_tiles[mi][:],
                        h_tiles[mi][:],
                        start=(mi == 0),
                        stop=(mi == 1),
                    )
                o_sb = opool.tile([OUT, CHUNK], f32, tag="osb")
                nc.scalar.activation(o_sb[:], o_ps[:], ident, bias=b1_t[:])
                n0 = c * CHUNK
                nc.sync.dma_start(out[:, n0:n0 + CHUNK], o_sb[:])

    nc.compile()
    return nc


_program_cache = {}


def _get_program():
    key = (MM_DT, CHUNK)
    if key not in _program_cache:
        _program_cache[key] = build_program()
    return _program_cache[key]


def kernel(**inputs: np.ndarray) -> np.ndarray:
    x = np.asarray(inputs["x"], dtype=np.float32)
    conv_w = np.asarray(inputs["conv_w"], dtype=np.float32)
    w0 = np.asarray(inputs["w0"], dtype=np.float32)
    b0 = np.asarray(inputs["b0"], dtype=np.float32)
    w1 = np.asarray(inputs["w1"], dtype=np.float32)
    b1 = np.asarray(inputs["b1"], dtype=np.float32)

    w_eff = fold_conv_into_fc(conv_w, w0)
    w0t_np = np.ascontiguousarray(w_eff.T)           # [784, 200]
    w1t_np = np.ascontiguousarray(w1.T)              # [200, 10]
    b0_np = np.ascontiguousarray(b0.reshape(HID, 1))
    b1_np = np.ascontiguousarray(b1.reshape(OUT, 1))

    in_maps = []
    for i in range(N_CORES):
        xm, xr = shard_layout(x[i * SHARD:(i + 1) * SHARD])
        in_maps.append({
            "xm": xm,
            "xr": xr,
            "w0t": w0t_np,
            "b0": b0_np,
            "w1t": w1t_np,
            "b1": b1_np,
        })

    nc = _get_program()

    profile = os.environ.get("BASS_KERNEL_PROFILE", "0") == "1"
    kwargs = {}
    if profile:
        _install_ntff_hook()
        kwargs = dict(trace=True, tmpdir=os.environ.get("BASS_KERNEL_TRACE_DIR"))
    res = run_bass_kernel_spmd(nc, in_maps, core_ids=list(range(N_CORES)), **kwargs)

    global last_exec_time_ns
    last_exec_time_ns = res.exec_time_ns

    out = np.empty((B, OUT), dtype=np.float32)
    for i in range(N_CORES):
        out[i * SHARD:(i + 1) * SHARD] = res.results[i]["out"].T
    return out


# revision 10
# speedup vs baseline: 1.8380x; 1.0124x over previous
"""Trainium2 Bass kernel for DigitConvolutionalModel forward pass.

Model: x[B,784] -> 3x3 valid conv (28x28 -> 26x26) -> flatten[676]
       -> Linear(676->200) + ReLU -> Linear(200->10).

Key algebraic optimization: the conv is linear and feeds straight into the
first Linear, so both fold into a single effective weight
W_eff[200,784] = w0 compose conv  (computed once on host, ~1.2 MFLOP).
The device then runs two dense GEMMs per batch shard:
    h = relu(x @ W_eff.T + b0);  out = h @ w1.T + b1

Sharding: pure data parallel over the batch dim across 8 NeuronCores
(4096 rows each). Weights are replicated. No collectives needed (forward
only). Everything on-device runs in a feature-major ("transposed") layout
so the contraction dim always lives on SBUF partitions:
    xT[784,4096] -> hT[200,4096] -> outT[10,4096]
The host pre-transposes x shards (and pre-tiles them so every DMA reads
long contiguous runs per SBUF partition) and post-transposes the tiny
output.
"""

import os
import sys
import types
import numpy as np

for _p in ("/opt/trn_rl_repo", "/root/.axon_site"):
    if os.path.isdir(_p) and _p not in sys.path:
        sys.path.insert(0, _p)

import concourse.bass as bass  # noqa: E402
import concourse.tile as tile  # noqa: E402
import concourse.mybir as mybir  # noqa: E402
from concourse import bacc  # noqa: E402
from concourse.bass_utils import run_bass_kernel_spmd  # noqa: E402

B = 32768
N_CORES = 8
SHARD = B // N_CORES          # 4096
KDIM = 784                    # 28*28 input features (conv folded in)
HID = 200
OUT = 10
CHUNK = 512                   # batch columns per matmul (moving free dim)
K_FULL = KDIM // 128          # 6 full 128-partition k-tiles
K_REM = KDIM - K_FULL * 128   # 16 remainder rows
M_TILES = [(0, 128), (128, 72)]  # hidden 200 = 128 + 72 PSUM partition tiles

# matmul operand dtype: float32 (exact, 4 cyc/row) or float32r
# (1 cyc/row at N>=256, reduced-precision multiplies, rel err ~2e-4)
MM_DT = mybir.dt.float32r

last_exec_time_ns = None      # set when BASS_KERNEL_PROFILE=1


def _install_ntff_hook():
    """Register the axon NTFF profile hook if the image's antenv lacks it."""
    try:
        from antenv.axon_hooks import get_axon_ntff_profile_hook  # noqa: F401
        return
    except ImportError:
        pass
    try:
        from trn_agent_boot.trn_boot import _ntff_profile_via_ctypes
        hook = _ntff_profile_via_ctypes("/opt/axon/libaxon_pjrt.so")
    except Exception:
        hook = None
    mod = types.ModuleType("antenv.axon_hooks")
    mod.get_axon_ntff_profile_hook = lambda: hook
    mod.set_axon_ntff_profile_hook = lambda h: None
    sys.modules["antenv.axon_hooks"] = mod


def fold_conv_into_fc(conv_w: np.ndarray, w0: np.ndarray) -> np.ndarray:
    """W_eff[200,784] such that x @ W_eff.T == fc1(flatten(conv(x)))."""
    w0v = w0.reshape(HID, 26, 26).astype(np.float64)
    w_img = np.zeros((HID, 28, 28), dtype=np.float64)
    for ki in range(3):
        for kj in range(3):
            w_img[:, ki:ki + 26, kj:kj + 26] += w0v * np.float64(conv_w[ki, kj])
    return w_img.reshape(HID, KDIM).astype(np.float32)


def shard_layout(xs: np.ndarray):
    """Pre-tile one x shard [4096, 784] for DMA-friendly loads.

    Returns (xm, xr):
      xm[c, p, a, n] = x[c*CHUNK+n, a*128+p]   shape [8, 128, 6, 512]
      xr[c, p, n]    = x[c*CHUNK+n, 768+p]     shape [8, 16, 512]
    so each SBUF partition line is one contiguous (a, n) run of 12 KB.
    """
    n_chunks = SHARD // CHUNK
    xsv = xs.reshape(n_chunks, CHUNK, KDIM)
    xm = np.ascontiguousarray(
        xsv[:, :, :K_FULL * 128]
        .reshape(n_chunks, CHUNK, K_FULL, 128)
        .transpose(0, 3, 2, 1)
    )
    xr = np.ascontiguousarray(
        xsv[:, :, K_FULL * 128:].transpose(0, 2, 1)
    )
    return xm, xr


def build_program():
    nc = bacc.Bacc("TRN2", target_bir_lowering=False, debug=False)
    f32 = mybir.dt.float32
    n_chunks = SHARD // CHUNK

    xm_d = nc.declare_dram_parameter(
        "xm", [n_chunks, 128, K_FULL, CHUNK], MM_DT, isOutput=False)
    xr_d = nc.declare_dram_parameter(
        "xr", [n_chunks, K_REM, CHUNK], MM_DT, isOutput=False)
    w0t = nc.declare_dram_parameter("w0t", [KDIM, HID], MM_DT, isOutput=False)
    b0 = nc.declare_dram_parameter("b0", [HID, 1], f32, isOutput=False)
    w1t = nc.declare_dram_parameter("w1t", [HID, OUT], MM_DT, isOutput=False)
    b1 = nc.declare_dram_parameter("b1", [OUT, 1], f32, isOutput=False)
    out = nc.declare_dram_parameter("out", [OUT, SHARD], f32, isOutput=True)

    relu = mybir.ActivationFunctionType.Relu
    ident = mybir.ActivationFunctionType.Identity

    with tile.TileContext(nc) as tc:
        with (
            tc.tile_pool(name="weights", bufs=1) as wpool,
            tc.tile_pool(name="xin", bufs=5) as xpool,
            tc.tile_pool(name="hbuf", bufs=2) as hpool,
            tc.tile_pool(name="obuf", bufs=4) as opool,
            tc.tile_pool(name="psum", bufs=2, space=bass.MemorySpace.PSUM) as pp,
            tc.tile_pool(name="opsum", bufs=2, space=bass.MemorySpace.PSUM) as op,
        ):
            # ---- replicated weights, loaded once (SWDGE; keeps the two
            # HWDGE rings free for the first x-chunk loads) ----
            w0_main = wpool.tile([128, K_FULL, HID], MM_DT)   # k rows 0..767
            nc.sync.dma_start(
                w0_main[:],
                w0t[0:K_FULL * 128, :].rearrange("(a p) m -> p a m", p=128),
            )
            w0_rem = wpool.tile([K_REM, HID], MM_DT)          # k rows 768..783
            nc.sync.dma_start(w0_rem[:], w0t[K_FULL * 128:KDIM, :])
            w1_a = wpool.tile([128, OUT], MM_DT)
            nc.sync.dma_start(w1_a[:], w1t[0:128, :])
            w1_b = wpool.tile([72, OUT], MM_DT)
            nc.sync.dma_start(w1_b[:], w1t[128:HID, :])
            b0_a = wpool.tile([128, 1], f32)
            nc.sync.dma_start(b0_a[:], b0[0:128, :])
            b0_b = wpool.tile([72, 1], f32)
            nc.sync.dma_start(b0_b[:], b0[128:HID, :])
            b1_t = wpool.tile([OUT, 1], f32)
            nc.sync.dma_start(b1_t[:], b1[:])
            b0_tiles = [b0_a, b0_b]
            w1_tiles = [w1_a, w1_b]

            for c in range(n_chunks):
                # alternate x loads across the two HWDGE rings (SP / ACT)
                dma_eng = nc.sync if c % 2 == 0 else nc.scalar
                xk = xpool.tile([128, K_FULL, CHUNK], MM_DT, tag="xk")
                dma_eng.dma_start(xk[:], xm_d[c])
                xr = xpool.tile([K_REM, CHUNK], MM_DT, tag="xr")
                dma_eng.dma_start(xr[:], xr_d[c])

                # layer 1: hT[m0:m0+dm, chunk] accumulated over 7 k-tiles
                h_tiles = []
                for mi, (m0, dm) in enumerate(M_TILES):
                    h_ps = pp.tile([dm, CHUNK], f32, tag=f"hps{mi}",
                                   name=f"hps_{c}_{mi}")
                    for a in range(K_FULL):
                        nc.tensor.matmul(
                            h_ps[:],
                            w0_main[:, a, m0:m0 + dm],
                            xk[:, a, :],
                            start=(a == 0),
                            stop=False,
                        )
                    nc.tensor.matmul(
                        h_ps[:],
                        w0_rem[:, m0:m0 + dm],
                        xr[:],
                        start=False,
                        stop=True,
                    )
                    h_sb = hpool.tile([dm, CHUNK], MM_DT, tag=f"h{mi}")
                    nc.scalar.activation(
                        h_sb[:], h_ps[:], relu, bias=b0_tiles[mi][:])
                    h_tiles.append(h_sb)

                # layer 2: outT[10, chunk] accumulated over 2 k-tiles
                o_ps = op.tile([OUT, CHUNK], f32, tag="ops")
                for mi in range(2):
                    nc.tensor.matmul(
                        o_ps[:],
                        w1_tiles[mi][:],
                        h_tiles[mi][:],
                        start=(mi == 0),
                        stop=(mi == 1),
                    )
                o_sb = opool.tile([OUT, CHUNK], f32, tag="osb")
                nc.scalar.activation(o_sb[:], o_ps[:], ident, bias=b1_t[:])
                n0 = c * CHUNK
                nc.sync.dma_start(out[:, n0:n0 + CHUNK], o_sb[:])

    nc.compile()
    return nc


_program_cache = {}


def _get_program():
    key = (MM_DT, CHUNK)
    if key not in _program_cache:
        _program_cache[key] = build_program()
    return _program_cache[key]


def kernel(**inputs: np.ndarray) -> np.ndarray:
    x = np.asarray(inputs["x"], dtype=np.float32)
    conv_w = np.asarray(inputs["conv_w"], dtype=np.float32)
    w0 = np.asarray(inputs["w0"], dtype=np.float32)
    b0 = np.asarray(inputs["b0"], dtype=np.float32)
    w1 = np.asarray(inputs["w1"], dtype=np.float32)
    b1 = np.asarray(inputs["b1"], dtype=np.float32)

    w_eff = fold_conv_into_fc(conv_w, w0)
    w0t_np = np.ascontiguousarray(w_eff.T)           # [784, 200]
    w1t_np = np.ascontiguousarray(w1.T)              # [200, 10]
    b0_np = np.ascontiguousarray(b0.reshape(HID, 1))
    b1_np = np.ascontiguousarray(b1.reshape(OUT, 1))

    in_maps = []
    for i in range(N_CORES):
        xm, xr = shard_layout(x[i * SHARD:(i + 1) * SHARD])
        in_maps.append({
            "xm": xm,
            "xr": xr,
            "w0t": w0t_np,
            "b0": b0_np,
            "w1t": w1t_np,
            "b1": b1_np,
        })

    nc = _get_program()

    profile = os.environ.get("BASS_KERNEL_PROFILE", "0") == "1"
    kwargs = {}
    if profile:
        _install_ntff_hook()
        kwargs = dict(trace=True, tmpdir=os.environ.get("BASS_KERNEL_TRACE_DIR"))
    res = run_bass_kernel_spmd(nc, in_maps, core_ids=list(range(N_CORES)), **kwargs)

    global last_exec_time_ns
    last_exec_time_ns = res.exec_time_ns

    out = np.empty((B, OUT), dtype=np.float32)
    for i in range(N_CORES):
        out[i * SHARD:(i + 1) * SHARD] = res.results[i]["out"].T
    return out


# revision 11
# speedup vs baseline: 2.0048x; 1.0908x over previous
"""Trainium2 Bass kernel for DigitConvolutionalModel forward pass.

Model: x[B,784] -> 3x3 valid conv (28x28 -> 26x26) -> flatten[676]
       -> Linear(676->200) + ReLU -> Linear(200->10).

Key algebraic optimization: the conv is linear and feeds straight into the
first Linear, so both fold into a single effective weight
W_eff[200,784] = w0 compose conv  (computed once on host, ~1.2 MFLOP).
The device then runs two dense GEMMs per batch shard:
    h = relu(x @ W_eff.T + b0);  out = h @ w1.T + b1

Sharding: pure data parallel over the batch dim across 8 NeuronCores
(4096 rows each). Weights are replicated. No collectives needed (forward
only). Everything on-device runs in a feature-major ("transposed") layout
so the contraction dim always lives on SBUF partitions:
    xT[784,4096] -> hT[200,4096] -> outT[10,4096]
The host pre-transposes x shards (and pre-tiles them so every DMA reads
long contiguous runs per SBUF partition) and post-transposes the tiny
output.
"""

import os
import sys
import types
import numpy as np

for _p in ("/opt/trn_rl_repo", "/root/.axon_site"):
    if os.path.isdir(_p) and _p not in sys.path:
        sys.path.insert(0, _p)

import concourse.bass as bass  # noqa: E402
import concourse.tile as tile  # noqa: E402
import concourse.mybir as mybir  # noqa: E402
from concourse import bacc  # noqa: E402
from concourse.bass_utils import run_bass_kernel_spmd  # noqa: E402

B = 32768
N_CORES = 8
SHARD = B // N_CORES          # 4096
KDIM = 784                    # 28*28 input features (conv folded in)
HID = 200
OUT = 10
CHUNK = 512                   # batch columns per matmul (moving free dim)
K_FULL = KDIM // 128          # 6 full 128-partition k-tiles
K_REM = KDIM - K_FULL * 128   # 16 remainder rows
M_TILES = [(0, 128), (128, 72)]  # hidden 200 = 128 + 72 PSUM partition tiles

# matmul operand dtype: float32 (exact, 4 cyc/row) or float32r
# (1 cyc/row at N>=256, reduced-precision multiplies, rel err ~2e-4)
MM_DT = mybir.dt.float32r

last_exec_time_ns = None      # set when BASS_KERNEL_PROFILE=1


def _install_ntff_hook():
    """Register the axon NTFF profile hook if the image's antenv lacks it."""
    try:
        from antenv.axon_hooks import get_axon_ntff_profile_hook  # noqa: F401
        return
    except ImportError:
        pass
    try:
        from trn_agent_boot.trn_boot import _ntff_profile_via_ctypes
        hook = _ntff_profile_via_ctypes("/opt/axon/libaxon_pjrt.so")
    except Exception:
        hook = None
    mod = types.ModuleType("antenv.axon_hooks")
    mod.get_axon_ntff_profile_hook = lambda: hook
    mod.set_axon_ntff_profile_hook = lambda h: None
    sys.modules["antenv.axon_hooks"] = mod


def fold_conv_into_fc(conv_w: np.ndarray, w0: np.ndarray) -> np.ndarray:
    """W_eff[200,784] such that x @ W_eff.T == fc1(flatten(conv(x)))."""
    w0v = w0.reshape(HID, 26, 26).astype(np.float64)
    w_img = np.zeros((HID, 28, 28), dtype=np.float64)
    for ki in range(3):
        for kj in range(3):
            w_img[:, ki:ki + 26, kj:kj + 26] += w0v * np.float64(conv_w[ki, kj])
    return w_img.reshape(HID, KDIM).astype(np.float32)


def shard_layout(xs: np.ndarray):
    """Pre-tile one x shard [4096, 784] for DMA-friendly loads.

    Returns (xm, xr):
      xm[c, p, a, n] = x[c*CHUNK+n, a*128+p]   shape [8, 128, 6, 512]
      xr[c, p, n]    = x[c*CHUNK+n, 768+p]     shape [8, 16, 512]
    so each SBUF partition line is one contiguous (a, n) run of 12 KB.
    """
    n_chunks = SHARD // CHUNK
    xsv = xs.reshape(n_chunks, CHUNK, KDIM)
    xm = np.ascontiguousarray(
        xsv[:, :, :K_FULL * 128]
        .reshape(n_chunks, CHUNK, K_FULL, 128)
        .transpose(0, 3, 2, 1)
    )
    xr = np.ascontiguousarray(
        xsv[:, :, K_FULL * 128:].transpose(0, 2, 1)
    )
    return xm, xr


def build_program():
    nc = bacc.Bacc("TRN2", target_bir_lowering=False, debug=False)
    f32 = mybir.dt.float32
    n_chunks = SHARD // CHUNK

    xm_d = nc.declare_dram_parameter(
        "xm", [n_chunks, 128, K_FULL, CHUNK], MM_DT, isOutput=False)
    xr_d = nc.declare_dram_parameter(
        "xr", [n_chunks, K_REM, CHUNK], MM_DT, isOutput=False)
    w0t = nc.declare_dram_parameter("w0t", [KDIM, HID], MM_DT, isOutput=False)
    b0 = nc.declare_dram_parameter("b0", [HID, 1], f32, isOutput=False)
    w1t = nc.declare_dram_parameter("w1t", [HID, OUT], MM_DT, isOutput=False)
    b1 = nc.declare_dram_parameter("b1", [OUT, 1], f32, isOutput=False)
    out = nc.declare_dram_parameter("out", [OUT, SHARD], f32, isOutput=True)

    relu = mybir.ActivationFunctionType.Relu
    ident = mybir.ActivationFunctionType.Identity

    HALF = K_FULL // 2  # 3 k-slices per x half-load
    with tile.TileContext(nc) as tc:
        with (
            tc.tile_pool(name="weights", bufs=1) as wpool,
            tc.tile_pool(name="xin", bufs=6) as xpool,
            tc.tile_pool(name="hbuf", bufs=2) as hpool,
            tc.tile_pool(name="obuf", bufs=4) as opool,
            tc.tile_pool(name="psum", bufs=2, space=bass.MemorySpace.PSUM) as pp,
            tc.tile_pool(name="opsum", bufs=2, space=bass.MemorySpace.PSUM) as op,
        ):
            # ---- replicated weights, one tile per k-slice so the first
            # matmuls only wait on small transfers; spread over both rings
            w0_a = []
            for a in range(K_FULL):
                w0s = wpool.tile([128, HID], MM_DT, name=f"w0s_{a}")
                eng = nc.sync if a % 2 == 0 else nc.scalar
                eng.dma_start(w0s[:], w0t[a * 128:(a + 1) * 128, :])
                w0_a.append(w0s)
            w0_rem = wpool.tile([K_REM, HID], MM_DT)          # k rows 768..783
            nc.sync.dma_start(w0_rem[:], w0t[K_FULL * 128:KDIM, :])
            w1_a = wpool.tile([128, OUT], MM_DT)
            nc.scalar.dma_start(w1_a[:], w1t[0:128, :])
            w1_b = wpool.tile([72, OUT], MM_DT)
            nc.sync.dma_start(w1_b[:], w1t[128:HID, :])
            b0_a = wpool.tile([128, 1], f32)
            nc.scalar.dma_start(b0_a[:], b0[0:128, :])
            b0_b = wpool.tile([72, 1], f32)
            nc.sync.dma_start(b0_b[:], b0[128:HID, :])
            b1_t = wpool.tile([OUT, 1], f32)
            nc.scalar.dma_start(b1_t[:], b1[:])
            b0_tiles = [b0_a, b0_b]
            w1_tiles = [w1_a, w1_b]

            for c in range(n_chunks):
                # each chunk's x feeds from BOTH HWDGE rings (SP + ACT)
                xh = []
                for h in range(2):
                    t = xpool.tile([128, HALF, CHUNK], MM_DT, tag=f"xk{h}",
                                   name=f"xk_{c}_{h}")
                    eng = nc.sync if (c + h) % 2 == 0 else nc.scalar
                    eng.dma_start(t[:], xm_d[c, :, h * HALF:(h + 1) * HALF, :])
                    xh.append(t)
                xr = xpool.tile([K_REM, CHUNK], MM_DT, tag="xr",
                                name=f"xr_{c}")
                (nc.sync if c % 2 == 0 else nc.scalar).dma_start(xr[:], xr_d[c])

                # layer 1: hT[m0:m0+dm, chunk] accumulated over 7 k-tiles
                h_tiles = []
                for mi, (m0, dm) in enumerate(M_TILES):
                    h_ps = pp.tile([dm, CHUNK], f32, tag=f"hps{mi}",
                                   name=f"hps_{c}_{mi}")
                    for a in range(K_FULL):
                        nc.tensor.matmul(
                            h_ps[:],
                            w0_a[a][:, m0:m0 + dm],
                            xh[a // HALF][:, a % HALF, :],
                            start=(a == 0),
                            stop=False,
                        )
                    nc.tensor.matmul(
                        h_ps[:],
                        w0_rem[:, m0:m0 + dm],
                        xr[:],
                        start=False,
                        stop=True,
                    )
                    h_sb = hpool.tile([dm, CHUNK], MM_DT, tag=f"h{mi}")
                    nc.scalar.activation(
                        h_sb[:], h_ps[:], relu, bias=b0_tiles[mi][:])
                    h_tiles.append(h_sb)

                # layer 2: outT[10, chunk] accumulated over 2 k-tiles
                o_ps = op.tile([OUT, CHUNK], f32, tag="ops")
                for mi in range(2):
                    nc.tensor.matmul(
                        o_ps[:],
                        w1_tiles[mi][:],
                        h_tiles[mi][:],
                        start=(mi == 0),
                        stop=(mi == 1),
                    )
                o_sb = opool.tile([OUT, CHUNK], f32, tag="osb")
                nc.scalar.activation(o_sb[:], o_ps[:], ident, bias=b1_t[:])
                n0 = c * CHUNK
                nc.sync.dma_start(out[:, n0:n0 + CHUNK], o_sb[:])

    nc.compile()
    return nc


_program_cache = {}


def _get_program():
    key = (MM_DT, CHUNK)
    if key not in _program_cache:
        _program_cache[key] = build_program()
    return _program_cache[key]


def kernel(**inputs: np.ndarray) -> np.ndarray:
    x = np.asarray(inputs["x"], dtype=np.float32)
    conv_w = np.asarray(inputs["conv_w"], dtype=np.float32)
    w0 = np.asarray(inputs["w0"], dtype=np.float32)
    b0 = np.asarray(inputs["b0"], dtype=np.float32)
    w1 = np.asarray(inputs["w1"], dtype=np.float32)
    b1 = np.asarray(inputs["b1"], dtype=np.float32)

    w_eff = fold_conv_into_fc(conv_w, w0)
    w0t_np = np.ascontiguousarray(w_eff.T)           # [784, 200]
    w1t_np = np.ascontiguousarray(w1.T)              # [200, 10]
    b0_np = np.ascontiguousarray(b0.reshape(HID, 1))
    b1_np = np.ascontiguousarray(b1.reshape(OUT, 1))

    in_maps = []
    for i in range(N_CORES):
        xm, xr = shard_layout(x[i * SHARD:(i + 1) * SHARD])
        in_maps.append({
            "xm": xm,
            "xr": xr,
            "w0t": w0t_np,
            "b0": b0_np,
            "w1t": w1t_np,
            "b1": b1_np,
        })

    nc = _get_program()

    profile = os.environ.get("BASS_KERNEL_PROFILE", "0") == "1"
    kwargs = {}
    if profile:
        _install_ntff_hook()
        kwargs = dict(trace=True, tmpdir=os.environ.get("BASS_KERNEL_TRACE_DIR"))
    res = run_bass_kernel_spmd(nc, in_maps, core_ids=list(range(N_CORES)), **kwargs)

    global last_exec_time_ns
    last_exec_time_ns = res.exec_time_ns

    out = np.empty((B, OUT), dtype=np.float32)
    for i in range(N_CORES):
        out[i * SHARD:(i + 1) * SHARD] = res.results[i]["out"].T
    return out


# revision 13
# speedup vs baseline: 2.4584x; 1.2263x over previous
"""Trainium2 Bass kernel for DigitConvolutionalModel forward pass.

Model: x[B,784] -> 3x3 valid conv (28x28 -> 26x26) -> flatten[676]
       -> Linear(676->200) + ReLU -> Linear(200->10).

Key algebraic optimization: the conv is linear and feeds straight into the
first Linear, so both fold into a single effective weight
W_eff[200,784] = w0 compose conv  (computed once on host, ~1.2 MFLOP).
The device then runs two dense GEMMs per batch shard:
    h = relu(x @ W_eff.T + b0);  out = h @ w1.T + b1

Sharding: pure data parallel over the batch dim across 8 NeuronCores
(4096 rows each). Weights are replicated. No collectives needed (forward
only). Everything on-device runs in a feature-major ("transposed") layout
so the contraction dim always lives on SBUF partitions:
    xT[784,4096] -> hT[200,4096] -> outT[10,4096]
The host pre-transposes x shards (and pre-tiles them so every DMA reads
long contiguous runs per SBUF partition) and post-transposes the tiny
output.
"""

import os
import sys
import types
import numpy as np

for _p in ("/opt/trn_rl_repo", "/root/.axon_site"):
    if os.path.isdir(_p) and _p not in sys.path:
        sys.path.insert(0, _p)

import concourse.bass as bass  # noqa: E402
import concourse.tile as tile  # noqa: E402
import concourse.mybir as mybir  # noqa: E402
from concourse import bacc  # noqa: E402
from concourse.bass_utils import run_bass_kernel_spmd  # noqa: E402

B = 32768
N_CORES = 8
SHARD = B // N_CORES          # 4096
KDIM = 784                    # 28*28 input features (conv folded in)
HID = 200
OUT = 10
CHUNK = 512                   # batch columns per matmul (moving free dim)
K_FULL = KDIM // 128          # 6 full 128-partition k-tiles
K_REM = KDIM - K_FULL * 128   # 16 remainder rows
M_TILES = [(0, 128), (128, 72)]  # hidden 200 = 128 + 72 PSUM partition tiles

# matmul operand dtype:
#   float32  — exact, 4 cyc/row (measured ~143us total)
#   float32r — ~2-3 cyc/row, rel err ~2e-4 (measured ~71us)
#   bfloat16 — 1 cyc/row + half DMA bytes, rel err ~5e-3
MM_DT = mybir.dt.bfloat16

last_exec_time_ns = None      # set when BASS_KERNEL_PROFILE=1


def _install_ntff_hook():
    """Register the axon NTFF profile hook if the image's antenv lacks it."""
    try:
        from antenv.axon_hooks import get_axon_ntff_profile_hook  # noqa: F401
        return
    except ImportError:
        pass
    try:
        from trn_agent_boot.trn_boot import _ntff_profile_via_ctypes
        hook = _ntff_profile_via_ctypes("/opt/axon/libaxon_pjrt.so")
    except Exception:
        hook = None
    mod = types.ModuleType("antenv.axon_hooks")
    mod.get_axon_ntff_profile_hook = lambda: hook
    mod.set_axon_ntff_profile_hook = lambda h: None
    sys.modules["antenv.axon_hooks"] = mod


def fold_conv_into_fc(conv_w: np.ndarray, w0: np.ndarray) -> np.ndarray:
    """W_eff[200,784] such that x @ W_eff.T == fc1(flatten(conv(x)))."""
    w0v = w0.reshape(HID, 26, 26).astype(np.float64)
    w_img = np.zeros((HID, 28, 28), dtype=np.float64)
    for ki in range(3):
        for kj in range(3):
            w_img[:, ki:ki + 26, kj:kj + 26] += w0v * np.float64(conv_w[ki, kj])
    return w_img.reshape(HID, KDIM).astype(np.float32)


def shard_layout(xs: np.ndarray):
    """Pre-tile one x shard [4096, 784] for DMA-friendly loads.

    Returns (xm, xr):
      xm[c, p, a, n] = x[c*CHUNK+n, a*128+p]   shape [8, 128, 6, 512]
      xr[c, p, n]    = x[c*CHUNK+n, 768+p]     shape [8, 16, 512]
    so each SBUF partition line is one contiguous (a, n) run of 12 KB.
    """
    n_chunks = SHARD // CHUNK
    xsv = xs.reshape(n_chunks, CHUNK, KDIM)
    xm = np.ascontiguousarray(
        xsv[:, :, :K_FULL * 128]
        .reshape(n_chunks, CHUNK, K_FULL, 128)
        .transpose(0, 3, 2, 1)
    )
    xr = np.ascontiguousarray(
        xsv[:, :, K_FULL * 128:].transpose(0, 2, 1)
    )
    return xm, xr


def build_program():
    nc = bacc.Bacc("TRN2", target_bir_lowering=False, debug=False)
    f32 = mybir.dt.float32
    n_chunks = SHARD // CHUNK

    xm_d = nc.declare_dram_parameter(
        "xm", [n_chunks, 128, K_FULL, CHUNK], MM_DT, isOutput=False)
    xr_d = nc.declare_dram_parameter(
        "xr", [n_chunks, K_REM, CHUNK], MM_DT, isOutput=False)
    w0t = nc.declare_dram_parameter("w0t", [KDIM, HID], MM_DT, isOutput=False)
    b0 = nc.declare_dram_parameter("b0", [HID, 1], f32, isOutput=False)
    w1t = nc.declare_dram_parameter("w1t", [HID, OUT], MM_DT, isOutput=False)
    b1 = nc.declare_dram_parameter("b1", [OUT, 1], f32, isOutput=False)
    out = nc.declare_dram_parameter("out", [OUT, SHARD], f32, isOutput=True)

    relu = mybir.ActivationFunctionType.Relu
    ident = mybir.ActivationFunctionType.Identity

    HALF = K_FULL // 2  # 3 k-slices per x half-load
    with tile.TileContext(nc) as tc:
        with (
            tc.tile_pool(name="weights", bufs=1) as wpool,
            tc.tile_pool(name="xin", bufs=6) as xpool,
            tc.tile_pool(name="hbuf", bufs=2) as hpool,
            tc.tile_pool(name="obuf", bufs=4) as opool,
            tc.tile_pool(name="psum", bufs=2, space=bass.MemorySpace.PSUM) as pp,
            tc.tile_pool(name="opsum", bufs=2, space=bass.MemorySpace.PSUM) as op,
        ):
            # ---- replicated weights, one tile per k-slice so the first
            # matmuls only wait on small transfers; spread over both rings
            w0_a = []
            for a in range(K_FULL):
                w0s = wpool.tile([128, HID], MM_DT, name=f"w0s_{a}")
                eng = nc.sync if a % 2 == 0 else nc.scalar
                eng.dma_start(w0s[:], w0t[a * 128:(a + 1) * 128, :])
                w0_a.append(w0s)
            w0_rem = wpool.tile([K_REM, HID], MM_DT)          # k rows 768..783
            nc.sync.dma_start(w0_rem[:], w0t[K_FULL * 128:KDIM, :])
            w1_a = wpool.tile([128, OUT], MM_DT)
            nc.scalar.dma_start(w1_a[:], w1t[0:128, :])
            w1_b = wpool.tile([72, OUT], MM_DT)
            nc.sync.dma_start(w1_b[:], w1t[128:HID, :])
            b0_a = wpool.tile([128, 1], f32)
            nc.scalar.dma_start(b0_a[:], b0[0:128, :])
            b0_b = wpool.tile([72, 1], f32)
            nc.sync.dma_start(b0_b[:], b0[128:HID, :])
            b1_t = wpool.tile([OUT, 1], f32)
            nc.scalar.dma_start(b1_t[:], b1[:])
            b0_tiles = [b0_a, b0_b]
            w1_tiles = [w1_a, w1_b]

            for c in range(n_chunks):
                # each chunk's x feeds from BOTH HWDGE rings (SP + ACT)
                xh = []
                for h in range(2):
                    t = xpool.tile([128, HALF, CHUNK], MM_DT, tag=f"xk{h}",
                                   name=f"xk_{c}_{h}")
                    eng = nc.sync if (c + h) % 2 == 0 else nc.scalar
                    eng.dma_start(t[:], xm_d[c, :, h * HALF:(h + 1) * HALF, :])
                    xh.append(t)
                xr = xpool.tile([K_REM, CHUNK], MM_DT, tag="xr",
                                name=f"xr_{c}")
                (nc.sync if c % 2 == 0 else nc.scalar).dma_start(xr[:], xr_d[c])

                # layer 1: hT[m0:m0+dm, chunk] accumulated over 7 k-tiles
                h_tiles = []
                for mi, (m0, dm) in enumerate(M_TILES):
                    h_ps = pp.tile([dm, CHUNK], f32, tag=f"hps{mi}",
                                   name=f"hps_{c}_{mi}")
                    for a in range(K_FULL):
                        nc.tensor.matmul(
                            h_ps[:],
                            w0_a[a][:, m0:m0 + dm],
                            xh[a // HALF][:, a % HALF, :],
                            start=(a == 0),
                            stop=False,
                        )
                    nc.tensor.matmul(
                        h_ps[:],
                        w0_rem[:, m0:m0 + dm],
                        xr[:],
                        start=False,
                        stop=True,
                    )
                    h_sb = hpool.tile([dm, CHUNK], MM_DT, tag=f"h{mi}")
                    nc.scalar.activation(
                        h_sb[:], h_ps[:], relu, bias=b0_tiles[mi][:])
                    h_tiles.append(h_sb)

                # layer 2: outT[10, chunk] accumulated over 2 k-tiles
                o_ps = op.tile([OUT, CHUNK], f32, tag="ops")
                for mi in range(2):
                    nc.tensor.matmul(
                        o_ps[:],
                        w1_tiles[mi][:],
                        h_tiles[mi][:],
                        start=(mi == 0),
                        stop=(mi == 1),
                    )
                o_sb = opool.tile([OUT, CHUNK], f32, tag="osb")
                nc.scalar.activation(o_sb[:], o_ps[:], ident, bias=b1_t[:])
                n0 = c * CHUNK
                nc.sync.dma_start(out[:, n0:n0 + CHUNK], o_sb[:])

    nc.compile()
    return nc


_program_cache = {}


def _get_program():
    key = (MM_DT, CHUNK)
    if key not in _program_cache:
        _program_cache[key] = build_program()
    return _program_cache[key]


def kernel(**inputs: np.ndarray) -> np.ndarray:
    x = np.asarray(inputs["x"], dtype=np.float32)
    conv_w = np.asarray(inputs["conv_w"], dtype=np.float32)
    w0 = np.asarray(inputs["w0"], dtype=np.float32)
    b0 = np.asarray(inputs["b0"], dtype=np.float32)
    w1 = np.asarray(inputs["w1"], dtype=np.float32)
    b1 = np.asarray(inputs["b1"], dtype=np.float32)

    if MM_DT == mybir.dt.bfloat16:
        import ml_dtypes
        mm_np = np.dtype(ml_dtypes.bfloat16)
    else:
        mm_np = np.dtype(np.float32)

    w_eff = fold_conv_into_fc(conv_w, w0)
    w0t_np = np.ascontiguousarray(w_eff.T.astype(mm_np))   # [784, 200]
    w1t_np = np.ascontiguousarray(w1.T.astype(mm_np))      # [200, 10]
    b0_np = np.ascontiguousarray(b0.reshape(HID, 1))
    b1_np = np.ascontiguousarray(b1.reshape(OUT, 1))

    in_maps = []
    for i in range(N_CORES):
        xm, xr = shard_layout(x[i * SHARD:(i + 1) * SHARD])
        in_maps.append({
            "xm": xm.astype(mm_np),
            "xr": xr.astype(mm_np),
            "w0t": w0t_np,
            "b0": b0_np,
            "w1t": w1t_np,
            "b1": b1_np,
        })

    nc = _get_program()

    profile = os.environ.get("BASS_KERNEL_PROFILE", "0") == "1"
    kwargs = {}
    if profile:
        _install_ntff_hook()
        kwargs = dict(trace=True, tmpdir=os.environ.get("BASS_KERNEL_TRACE_DIR"))
    res = run_bass_kernel_spmd(nc, in_maps, core_ids=list(range(N_CORES)), **kwargs)

    global last_exec_time_ns
    last_exec_time_ns = res.exec_time_ns

    out = np.empty((B, OUT), dtype=np.float32)
    for i in range(N_CORES):
        out[i * SHARD:(i + 1) * SHARD] = res.results[i]["out"].T
    return out
